# revision 16
# baseline (speedup 1.0000x reference)
"""HMM forward-backward marginal (nn_HMM_EM) on 8 Trainium2 NeuronCores.

Batch (8192) is sharded across 8 cores (1024 each); tiny T/pi params are
replicated. The host precomputes the (scaled) emission softmax, gathers it by
token, and uploads per-core transposed emission tensors E (bf16). Per core,
in transposed (Z, B) layout with two pipelined 512-column batch chunks:
  recurse: beta^T <- T^T @ (e_t^T * beta^T)   11 steps, accumulated in PSUM
  reduce:  s = pi^T @ (e_0^T * beta^T)        (1, 1024) per core
Host post-processing: out = S*log(SCALE) - log(s).
Emissions are pre-scaled by SCALE=128 so all intermediates stay well inside
fp32 range (log s ~ -55 + 12*log 128 ~ +3).
"""

import sys

sys.path.insert(0, "/opt/trn_rl_repo")

import numpy as np
import ml_dtypes

Z = 256        # hidden states
X = 64         # emission symbols
S = 12         # sequence length
B = 8192       # total batch
NCORES = 8
BL = B // NCORES   # 1024 batch per core
NBF = 512          # matmul free-dim chunk (one PSUM bank of fp32)
NB = BL // NBF     # 2 batch chunks per core
SCALE = 128.0

BF16 = ml_dtypes.bfloat16

_CACHE: dict = {}


def _build_bass():
    import concourse.mybir as mybir
    from concourse import bacc
    from concourse.tile import TileContext

    DT = mybir.dt.bfloat16
    F32 = mybir.dt.float32

    nc = bacc.Bacc("TRN2", target_bir_lowering=False, debug=False)

    # E columns: ((t*NB + bc)*2 + m)*NBF + b, partition p = z % 128, m = z // 128
    E = nc.dram_tensor("E", [128, S * NB * 2 * NBF], DT, kind="ExternalInput")
    # params packed into one tensor: cols [k*256+j] = T[k*128+p, j] for
    # k-chunk k, cols [512+k] = pi[k*128+p]
    P2 = nc.dram_tensor("P2", [128, 2 * Z + 2], DT, kind="ExternalInput")
    out_s = nc.dram_tensor("out_s", [1, BL], F32, kind="ExternalOutput")

    ZT = 2 * NBF  # 1024: two z-chunks side by side in the free dim

    with TileContext(nc) as tc:
        with (
            tc.tile_pool(name="const", bufs=1) as const,
            tc.tile_pool(name="bsb", bufs=3) as bpool,
            tc.tile_pool(name="wsb", bufs=4) as wpool,
            tc.tile_pool(name="osb", bufs=2) as opool,
            tc.tile_pool(name="pse", bufs=2, space="PSUM") as pse,
            tc.tile_pool(name="psb", bufs=1, space="PSUM") as psb,
        ):
            P_sb = const.tile([128, 2 * Z + 2], DT, name="P2")
            warm_sb = const.tile([128, 64], DT, name="warm")
            E_sb = [const.tile([128, NB * ZT], DT, name=f"E{t}") for t in range(S)]

            def T_lhsT(k, m):
                return P_sb[:, k * Z + m * 128 : k * Z + (m + 1) * 128]

            def pi_lhsT(k):
                return P_sb[:, 2 * Z + k : 2 * Z + k + 1]

            # Input DMAs split over the two hardware DGE rings (sync + scalar)
            # so descriptor issue runs in parallel: params and the first E
            # chunk lead their respective queues, the rest follow in
            # consumption order (t = S-1 first).
            nc.sync.dma_start(out=P_sb[:], in_=P2[:])
            t_last = S - 1
            # first chunk split at k-granularity: the first matmul pair only
            # needs the k=0 half of E_{S-1} bc0
            for h in range(2):
                nc.scalar.dma_start(
                    out=E_sb[t_last][:, h * NBF : (h + 1) * NBF],
                    in_=E[:, t_last * NB * ZT + h * NBF : t_last * NB * ZT + (h + 1) * NBF],
                )
            nc.scalar.dma_start(
                out=E_sb[t_last][:, ZT : 2 * ZT],
                in_=E[:, (t_last * NB + 1) * ZT : (t_last * NB + 2) * ZT],
            )
            for i, t in enumerate(range(S - 2, -1, -1)):
                eng = nc.sync if i % 2 == 0 else nc.scalar
                eng.dma_start(
                    out=E_sb[t][:], in_=E[:, t * NB * ZT : (t + 1) * NB * ZT]
                )

            # PE pre-warm: short dummy matmuls bridge the input-DMA wait so
            # the tensor engine's activity monitor (HAM) sees continuous work
            # and the real matmuls run at full clock sooner.
            warm_ps = pse.tile([128, NBF], F32, name="sps")
            nc.gpsimd.memset(warm_sb[:], 1.0)
            for _ in range(18):
                nc.tensor.matmul(
                    warm_ps[0:64, 0:64],
                    warm_sb[:, 0:64],
                    warm_sb[:, 0:64],
                    start=True,
                    stop=True,
                )

            # persistent 2-bank PSUM accumulators for beta^T, one per batch
            # chunk; columns [m*NBF,(m+1)*NBF) hold z-chunk m.
            beta_ps = [psb.tile([128, ZT], F32, name=f"beta{bc}") for bc in range(NB)]

            def e_slice(t, bc):
                return E_sb[t][:, bc * ZT : (bc + 1) * ZT]

            for t in range(S - 1, -1, -1):
                for bc in range(NB):
                    if t == S - 1:
                        w = e_slice(t, bc)  # beta starts at ones: w = e
                    else:
                        # multiply straight out of PSUM (1x mode)
                        wt = wpool.tile([128, ZT], DT, name="wsb")
                        nc.vector.tensor_mul(
                            out=wt[:], in0=e_slice(t, bc), in1=beta_ps[bc][:]
                        )
                        w = wt[:]

                    if t > 0:
                        for m in range(2):
                            for k in range(2):
                                nc.tensor.matmul(
                                    beta_ps[bc][:, m * NBF : (m + 1) * NBF],
                                    T_lhsT(k, m),
                                    w[:, k * NBF : (k + 1) * NBF],
                                    start=(k == 0),
                                    stop=(k == 1),
                                )
                    else:
                        s_ps = pse.tile([128, NBF], F32, name="sps")
                        for k in range(2):
                            nc.tensor.matmul(
                                s_ps[0:1, :],
                                pi_lhsT(k),
                                w[:, k * NBF : (k + 1) * NBF],
                                start=(k == 0),
                                stop=(k == 1),
                            )
                        s_sb = opool.tile([1, NBF], F32, name="ssb")
                        nc.vector.tensor_copy(out=s_sb[:], in_=s_ps[0:1, :])
                        nc.sync.dma_start(
                            out=out_s[0:1, bc * NBF : (bc + 1) * NBF], in_=s_sb[:]
                        )

    nc.compile()
    return nc


def _get_nc():
    if "nc" not in _CACHE:
        _CACHE["nc"] = _build_bass()
    return _CACHE["nc"]


def _softmax0(x):
    x = np.asarray(x, np.float32)
    m = x.max(axis=0, keepdims=True)
    e = np.exp(x - m)
    return e / e.sum(axis=0, keepdims=True)


def _prepare_in_maps(tokens, T_logits, pi_logits, emit_logits):
    tokens = np.asarray(tokens).astype(np.int32)
    T = _softmax0(T_logits)                      # (Z, Z) columns sum to 1
    pi = _softmax0(pi_logits)                    # (Z,)
    emit = _softmax0(emit_logits) * np.float32(SCALE)  # (X, Z), pre-scaled

    # packed params: P2[p, k*256+j] = T[k*128+p, j]; P2[p, 512+k] = pi[k*128+p]
    P2 = np.zeros((128, 2 * Z + 2), np.float32)
    P2[:, 0:Z] = T[0:128, :]
    P2[:, Z : 2 * Z] = T[128:256, :]
    P2[:, 2 * Z : 2 * Z + 2] = pi.reshape(2, 128).T
    P2 = P2.astype(BF16)

    # Pre-gathered emissions, transposed per-core:
    # E[core][p, ((t*NB + bc)*2 + m)*NBF + b] = emit[tokens[t, g]] * SCALE
    # with g = core*BL + bc*NBF + b and z = m*128 + p.
    e_all = emit[tokens].astype(BF16)            # (S, B, Z)
    E = (
        e_all.reshape(S, NCORES, NB, NBF, 2, 128)
        .transpose(1, 5, 0, 2, 4, 3)
        .reshape(NCORES, 128, S * NB * 2 * NBF)
    )
    E = np.ascontiguousarray(E)

    return [{"E": E[c], "P2": P2} for c in range(NCORES)]


def _run(inputs, trace=False, tmpdir=None):
    from concourse.bass_utils import run_bass_kernel_spmd

    in_maps = _prepare_in_maps(
        inputs["tokens"],
        inputs["T_logits"],
        inputs["pi_logits"],
        inputs["emit_logits"],
    )
    nc = _get_nc()
    res = run_bass_kernel_spmd(
        nc, in_maps, list(range(NCORES)), trace=trace, tmpdir=tmpdir
    )
    s = np.concatenate(
        [res.results[c]["out_s"].reshape(-1) for c in range(NCORES)]
    ).astype(np.float32)
    out = np.float32(S * np.log(SCALE)) - np.log(s)
    return out.astype(np.float32), res


def kernel(**inputs):
    return _run(inputs, trace=False)[0]


# revision 19
# speedup vs baseline: 1.0010x; 1.0010x over previous
"""HMM forward-backward marginal (nn_HMM_EM) on 8 Trainium2 NeuronCores.

Batch (8192) is sharded across 8 cores (1024 each); tiny T/pi params are
replicated. The host precomputes the (scaled) emission softmax, gathers it by
token, and uploads per-core transposed emission tensors E (bf16). Per core,
in transposed (Z, B) layout with two pipelined 512-column batch chunks:
  recurse: beta^T <- T^T @ (e_t^T * beta^T)   11 steps, accumulated in PSUM
  reduce:  s = pi^T @ (e_0^T * beta^T)        (1, 1024) per core
Host post-processing: out = S*log(SCALE) - log(s).
Emissions are pre-scaled by SCALE=128 so all intermediates stay well inside
fp32 range (log s ~ -55 + 12*log 128 ~ +3).
"""

import sys

sys.path.insert(0, "/opt/trn_rl_repo")

import numpy as np
import ml_dtypes

Z = 256        # hidden states
X = 64         # emission symbols
S = 12         # sequence length
B = 8192       # total batch
NCORES = 8
BL = B // NCORES   # 1024 batch per core
NBF = 512          # matmul free-dim chunk (one PSUM bank of fp32)
NB = BL // NBF     # 2 batch chunks per core
SCALE = 128.0

BF16 = ml_dtypes.bfloat16

_CACHE: dict = {}


def _build_bass():
    import concourse.mybir as mybir
    from concourse import bacc
    from concourse.tile import TileContext

    DT = mybir.dt.bfloat16
    F32 = mybir.dt.float32

    nc = bacc.Bacc("TRN2", target_bir_lowering=False, debug=False)

    # E columns: ((t*NB + bc)*2 + m)*NBF + b, partition p = z % 128, m = z // 128
    E = nc.dram_tensor("E", [128, S * NB * 2 * NBF], DT, kind="ExternalInput")
    # params packed into one tensor: cols [k*256+j] = T[k*128+p, j] for
    # k-chunk k, cols [512+k] = pi[k*128+p]
    P2 = nc.dram_tensor("P2", [128, 2 * Z + 2], DT, kind="ExternalInput")
    out_s = nc.dram_tensor("out_s", [1, BL], F32, kind="ExternalOutput")

    ZT = 2 * NBF  # 1024: two z-chunks side by side in the free dim

    with TileContext(nc) as tc:
        with (
            tc.tile_pool(name="const", bufs=1) as const,
            tc.tile_pool(name="bsb", bufs=3) as bpool,
            tc.tile_pool(name="wsb", bufs=4) as wpool,
            tc.tile_pool(name="osb", bufs=2) as opool,
            tc.tile_pool(name="pse", bufs=2, space="PSUM") as pse,
            tc.tile_pool(name="psb", bufs=1, space="PSUM") as psb,
        ):
            P_sb = const.tile([128, 2 * Z + 2], DT, name="P2")
            warm_sb = const.tile([128, 64], DT, name="warm")
            warm_rhs = const.tile([128, NBF], DT, name="warmr")
            E_sb = [const.tile([128, NB * ZT], DT, name=f"E{t}") for t in range(S)]

            def T_lhsT(k, m):
                return P_sb[:, k * Z + m * 128 : k * Z + (m + 1) * 128]

            def pi_lhsT(k):
                return P_sb[:, 2 * Z + k : 2 * Z + k + 1]

            # Input DMAs split over the two hardware DGE rings (sync + scalar)
            # so descriptor issue runs in parallel: params and the first E
            # chunk lead their respective queues, the rest follow in
            # consumption order (t = S-1 first).
            nc.sync.dma_start(out=P_sb[:], in_=P2[:])
            t_last = S - 1
            for bc in range(NB):
                nc.scalar.dma_start(
                    out=E_sb[t_last][:, bc * ZT : (bc + 1) * ZT],
                    in_=E[:, (t_last * NB + bc) * ZT : (t_last * NB + bc + 1) * ZT],
                )
            for i, t in enumerate(range(S - 2, -1, -1)):
                eng = nc.sync if i % 2 == 0 else nc.scalar
                eng.dma_start(
                    out=E_sb[t][:], in_=E[:, t * NB * ZT : (t + 1) * NB * ZT]
                )

            # PE pre-warm: dummy matmuls bridge the input-DMA wait so the
            # tensor engine's activity monitor (HAM) sees continuous work and
            # the real matmuls run at full clock from the start.
            warm_ps = pse.tile([128, NBF], F32, name="sps")
            nc.gpsimd.memset(warm_sb[:], 1.0)
            nc.gpsimd.memset(warm_rhs[:], 1.0)
            for _ in range(9):
                nc.tensor.matmul(
                    warm_ps[0:64, :],
                    warm_sb[:, 0:64],
                    warm_rhs[:],
                    start=True,
                    stop=True,
                )

            # persistent 2-bank PSUM accumulators for beta^T, one per batch
            # chunk; columns [m*NBF,(m+1)*NBF) hold z-chunk m.
            beta_ps = [psb.tile([128, ZT], F32, name=f"beta{bc}") for bc in range(NB)]

            def e_slice(t, bc):
                return E_sb[t][:, bc * ZT : (bc + 1) * ZT]

            for t in range(S - 1, -1, -1):
                for bc in range(NB):
                    if t == S - 1:
                        w = e_slice(t, bc)  # beta starts at ones: w = e
                    else:
                        # multiply straight out of PSUM (1x mode)
                        wt = wpool.tile([128, ZT], DT, name="wsb")
                        nc.vector.tensor_mul(
                            out=wt[:], in0=e_slice(t, bc), in1=beta_ps[bc][:]
                        )
                        w = wt[:]

                    if t > 0:
                        for m in range(2):
                            for k in range(2):
                                nc.tensor.matmul(
                                    beta_ps[bc][:, m * NBF : (m + 1) * NBF],
                                    T_lhsT(k, m),
                                    w[:, k * NBF : (k + 1) * NBF],
                                    start=(k == 0),
                                    stop=(k == 1),
                                )
                    else:
                        s_ps = pse.tile([128, NBF], F32, name="sps")
                        for k in range(2):
                            nc.tensor.matmul(
                                s_ps[0:1, :],
                                pi_lhsT(k),
                                w[:, k * NBF : (k + 1) * NBF],
                                start=(k == 0),
                                stop=(k == 1),
                            )
                        s_sb = opool.tile([1, NBF], F32, name="ssb")
                        nc.vector.tensor_copy(out=s_sb[:], in_=s_ps[0:1, :])
                        nc.sync.dma_start(
                            out=out_s[0:1, bc * NBF : (bc + 1) * NBF], in_=s_sb[:]
                        )

    nc.compile()
    return nc


def _get_nc():
    if "nc" not in _CACHE:
        _CACHE["nc"] = _build_bass()
    return _CACHE["nc"]


def _softmax0(x):
    x = np.asarray(x, np.float32)
    m = x.max(axis=0, keepdims=True)
    e = np.exp(x - m)
    return e / e.sum(axis=0, keepdims=True)


def _prepare_in_maps(tokens, T_logits, pi_logits, emit_logits):
    tokens = np.asarray(tokens).astype(np.int32)
    T = _softmax0(T_logits)                      # (Z, Z) columns sum to 1
    pi = _softmax0(pi_logits)                    # (Z,)
    emit = _softmax0(emit_logits) * np.float32(SCALE)  # (X, Z), pre-scaled

    # packed params: P2[p, k*256+j] = T[k*128+p, j]; P2[p, 512+k] = pi[k*128+p]
    P2 = np.zeros((128, 2 * Z + 2), np.float32)
    P2[:, 0:Z] = T[0:128, :]
    P2[:, Z : 2 * Z] = T[128:256, :]
    P2[:, 2 * Z : 2 * Z + 2] = pi.reshape(2, 128).T
    P2 = P2.astype(BF16)

    # Pre-gathered emissions, transposed per-core:
    # E[core][p, ((t*NB + bc)*2 + m)*NBF + b] = emit[tokens[t, g]] * SCALE
    # with g = core*BL + bc*NBF + b and z = m*128 + p.
    e_all = emit[tokens].astype(BF16)            # (S, B, Z)
    E = (
        e_all.reshape(S, NCORES, NB, NBF, 2, 128)
        .transpose(1, 5, 0, 2, 4, 3)
        .reshape(NCORES, 128, S * NB * 2 * NBF)
    )
    E = np.ascontiguousarray(E)

    return [{"E": E[c], "P2": P2} for c in range(NCORES)]


def _run(inputs, trace=False, tmpdir=None):
    from concourse.bass_utils import run_bass_kernel_spmd

    in_maps = _prepare_in_maps(
        inputs["tokens"],
        inputs["T_logits"],
        inputs["pi_logits"],
        inputs["emit_logits"],
    )
    nc = _get_nc()
    res = run_bass_kernel_spmd(
        nc, in_maps, list(range(NCORES)), trace=trace, tmpdir=tmpdir
    )
    s = np.concatenate(
        [res.results[c]["out_s"].reshape(-1) for c in range(NCORES)]
    ).astype(np.float32)
    out = np.float32(S * np.log(SCALE)) - np.log(s)
    return out.astype(np.float32), res


def kernel(**inputs):
    return _run(inputs, trace=False)[0]


# revision 23
# speedup vs baseline: 1.1066x; 1.1055x over previous
"""HMM forward-backward marginal (nn_HMM_EM) on 8 Trainium2 NeuronCores.

Batch (8192) is sharded across 8 cores (1024 each); tiny T/pi params are
replicated. The host precomputes the (scaled) emission softmax, gathers it by
token, and uploads per-core transposed emission tensors E (bf16). Per core,
in transposed (Z, B) layout with two pipelined 512-column batch chunks:
  recurse: beta^T <- T^T @ (e_t^T * beta^T)   11 steps, accumulated in PSUM
  reduce:  s = pi^T @ (e_0^T * beta^T)        (1, 1024) per core
Host post-processing: out = S*log(SCALE) - log(s).
Emissions are pre-scaled by SCALE=128 so all intermediates stay well inside
fp32 range (log s ~ -55 + 12*log 128 ~ +3).
"""

import sys

sys.path.insert(0, "/opt/trn_rl_repo")

import numpy as np
import ml_dtypes

Z = 256        # hidden states
X = 64         # emission symbols
S = 12         # sequence length
B = 8192       # total batch
NCORES = 8
BL = B // NCORES   # 1024 batch per core
NBF = 512          # matmul free-dim chunk (one PSUM bank of fp32)
NB = BL // NBF     # 2 batch chunks per core
SCALE = 128.0

BF16 = ml_dtypes.bfloat16

_CACHE: dict = {}


def _build_bass():
    import concourse.mybir as mybir
    from concourse import bacc
    from concourse.tile import TileContext

    DT = mybir.dt.bfloat16
    F32 = mybir.dt.float32

    nc = bacc.Bacc("TRN2", target_bir_lowering=False, debug=False)

    F8 = mybir.dt.float8e4
    ZT_ = 2 * NBF
    # E columns: ((t*NB + bc)*2 + m)*NBF + b, partition p = z % 128, m = z // 128
    # E11 (feeds matmuls directly) stays bf16; earlier steps ship as fp8 —
    # they only feed the elementwise multiply, whose output is bf16.
    E11 = nc.dram_tensor("E11", [128, NB * ZT_], DT, kind="ExternalInput")
    Ef = nc.dram_tensor("Ef", [128, (S - 1) * NB * ZT_], F8, kind="ExternalInput")
    # params packed into one tensor: cols [k*256+j] = T[k*128+p, j] for
    # k-chunk k, cols [512+k] = pi[k*128+p]
    P2 = nc.dram_tensor("P2", [128, 2 * Z + 2], DT, kind="ExternalInput")
    out_s = nc.dram_tensor("out_s", [1, BL], F32, kind="ExternalOutput")

    ZT = 2 * NBF  # 1024: two z-chunks side by side in the free dim

    with TileContext(nc) as tc:
        with (
            tc.tile_pool(name="const", bufs=1) as const,
            tc.tile_pool(name="bsb", bufs=3) as bpool,
            tc.tile_pool(name="wsb", bufs=4) as wpool,
            tc.tile_pool(name="osb", bufs=2) as opool,
            tc.tile_pool(name="pse", bufs=2, space="PSUM") as pse,
            tc.tile_pool(name="psb", bufs=1, space="PSUM") as psb,
        ):
            P_sb = const.tile([128, 2 * Z + 2], DT, name="P2")
            warm_sb = const.tile([128, 64], DT, name="warm")
            warm_rhs = const.tile([128, NBF], DT, name="warmr")
            E_sb = [
                const.tile([128, NB * ZT], DT if t == S - 1 else F8, name=f"E{t}")
                for t in range(S)
            ]

            def T_lhsT(k, m):
                return P_sb[:, k * Z + m * 128 : k * Z + (m + 1) * 128]

            def pi_lhsT(k):
                return P_sb[:, 2 * Z + k : 2 * Z + k + 1]

            # Input DMAs split over the two hardware DGE rings (sync + scalar)
            # so descriptor issue runs in parallel: params and the first E
            # chunk lead their respective queues, the rest follow in
            # consumption order (t = S-1 first).
            nc.sync.dma_start(out=P_sb[:], in_=P2[:])
            t_last = S - 1
            for bc in range(NB):
                nc.scalar.dma_start(
                    out=E_sb[t_last][:, bc * ZT : (bc + 1) * ZT],
                    in_=E11[:, bc * ZT : (bc + 1) * ZT],
                )
            for i, t in enumerate(range(S - 2, -1, -1)):
                eng = nc.sync if i % 2 == 0 else nc.scalar
                eng.dma_start(
                    out=E_sb[t][:], in_=Ef[:, t * NB * ZT : (t + 1) * NB * ZT]
                )

            # PE pre-warm: dummy matmuls bridge the input-DMA wait so the
            # tensor engine's activity monitor (HAM) sees continuous work and
            # the real matmuls run at full clock from the start.
            warm_ps = pse.tile([128, NBF], F32, name="sps")
            nc.gpsimd.memset(warm_sb[:], 1.0)
            nc.gpsimd.memset(warm_rhs[:], 1.0)
            for _ in range(9):
                nc.tensor.matmul(
                    warm_ps[0:64, :],
                    warm_sb[:, 0:64],
                    warm_rhs[:],
                    start=True,
                    stop=True,
                )

            # persistent 2-bank PSUM accumulators for beta^T, one per batch
            # chunk; columns [m*NBF,(m+1)*NBF) hold z-chunk m.
            beta_ps = [psb.tile([128, ZT], F32, name=f"beta{bc}") for bc in range(NB)]

            def e_slice(t, bc):
                return E_sb[t][:, bc * ZT : (bc + 1) * ZT]

            for t in range(S - 1, -1, -1):
                for bc in range(NB):
                    if t == S - 1:
                        w = e_slice(t, bc)  # beta starts at ones: w = e
                    else:
                        # multiply straight out of PSUM (1x mode)
                        wt = wpool.tile([128, ZT], DT, name="wsb")
                        nc.vector.tensor_mul(
                            out=wt[:], in0=e_slice(t, bc), in1=beta_ps[bc][:]
                        )
                        w = wt[:]

                    if t > 0:
                        for m in range(2):
                            for k in range(2):
                                nc.tensor.matmul(
                                    beta_ps[bc][:, m * NBF : (m + 1) * NBF],
                                    T_lhsT(k, m),
                                    w[:, k * NBF : (k + 1) * NBF],
                                    start=(k == 0),
                                    stop=(k == 1),
                                )
                    else:
                        s_ps = pse.tile([128, NBF], F32, name="sps")
                        for k in range(2):
                            nc.tensor.matmul(
                                s_ps[0:1, :],
                                pi_lhsT(k),
                                w[:, k * NBF : (k + 1) * NBF],
                                start=(k == 0),
                                stop=(k == 1),
                            )
                        s_sb = opool.tile([1, NBF], F32, name="ssb")
                        nc.vector.tensor_copy(out=s_sb[:], in_=s_ps[0:1, :])
                        nc.sync.dma_start(
                            out=out_s[0:1, bc * NBF : (bc + 1) * NBF], in_=s_sb[:]
                        )

    nc.compile()
    return nc


def _get_nc():
    if "nc" not in _CACHE:
        _CACHE["nc"] = _build_bass()
    return _CACHE["nc"]


def _softmax0(x):
    x = np.asarray(x, np.float32)
    m = x.max(axis=0, keepdims=True)
    e = np.exp(x - m)
    return e / e.sum(axis=0, keepdims=True)


def _prepare_in_maps(tokens, T_logits, pi_logits, emit_logits):
    tokens = np.asarray(tokens).astype(np.int32)
    T = _softmax0(T_logits)                      # (Z, Z) columns sum to 1
    pi = _softmax0(pi_logits)                    # (Z,)
    emit = _softmax0(emit_logits) * np.float32(SCALE)  # (X, Z), pre-scaled

    # packed params: P2[p, k*256+j] = T[k*128+p, j]; P2[p, 512+k] = pi[k*128+p]
    P2 = np.zeros((128, 2 * Z + 2), np.float32)
    P2[:, 0:Z] = T[0:128, :]
    P2[:, Z : 2 * Z] = T[128:256, :]
    P2[:, 2 * Z : 2 * Z + 2] = pi.reshape(2, 128).T
    P2 = P2.astype(BF16)

    # Pre-gathered emissions, transposed per-core:
    # E[core][p, ((t*NB + bc)*2 + m)*NBF + b] = emit[tokens[t, g]] * SCALE
    # with g = core*BL + bc*NBF + b and z = m*128 + p.
    e_all = emit[tokens]                         # (S, B, Z) fp32
    E = (
        e_all.reshape(S, NCORES, NB, NBF, 2, 128)
        .transpose(1, 5, 0, 2, 4, 3)
        .reshape(NCORES, 128, S, NB * 2 * NBF)
    )
    E = np.ascontiguousarray(E)
    E11 = E[:, :, S - 1, :].astype(BF16)
    Ef = np.ascontiguousarray(
        E[:, :, 0 : S - 1, :].reshape(NCORES, 128, (S - 1) * NB * 2 * NBF)
    ).astype(ml_dtypes.float8_e4m3fn)

    return [{"E11": E11[c], "Ef": Ef[c], "P2": P2} for c in range(NCORES)]


def _run(inputs, trace=False, tmpdir=None):
    from concourse.bass_utils import run_bass_kernel_spmd

    in_maps = _prepare_in_maps(
        inputs["tokens"],
        inputs["T_logits"],
        inputs["pi_logits"],
        inputs["emit_logits"],
    )
    nc = _get_nc()
    res = run_bass_kernel_spmd(
        nc, in_maps, list(range(NCORES)), trace=trace, tmpdir=tmpdir
    )
    s = np.concatenate(
        [res.results[c]["out_s"].reshape(-1) for c in range(NCORES)]
    ).astype(np.float32)
    out = np.float32(S * np.log(SCALE)) - np.log(s)
    return out.astype(np.float32), res


def kernel(**inputs):
    return _run(inputs, trace=False)[0]
